# revision 30
# baseline (speedup 1.0000x reference)
"""MelSpectrogramNet on 8 TRN2 NeuronCores (Bass/Tile), data-parallel over batch.

Math (per batch item):
  stft[f,t]  = (sum_k x[256t+k]*wc[f,k])^2 + (sum_k x[256t+k]*ws[f,k])^2
  mel        = mel_w @ stft
  x_db       = 10*log10(max(mel, 1e-10));  x_db = max(x_db, max_all(x_db)-80)
  out        = (x_db + 25) / 80

Device mapping:
  - hop 256 = 2*128, window 2048 = 16*128: x is de-interleaved on the host by
    128-column parity into C2[r, par, u] = x[256u + 128par + r] so the moving
    operand of the DFT matmul for k-chunk h and frame tile [t0, t0+Tt) is the
    CONTIGUOUS slice C2[:, h%2, t0+h//2 : t0+h//2+Tt] (full-rate PE streaming).
  - f32r matmuls (full PE rate, ~1e-4 rel err) for the DFT (16 accumulating
    K=128 matmuls per f-chunk of 128 freqs) and the mel projection.
  - Nyquist bin (f=1024): the f=0 SIN row is exactly zero, so the sin weight
    matrix carries the Nyquist cos row there instead (sin row at Nyquist is
    sin(pi*t) = 0, dropped). Then stft[0] = cos_0^2 + nyq^2; the mel weight
    column for f=0 is swapped to mel_w[:,1024] and a K=1 rank-1 matmul with
    (mel_w[:,0]-mel_w[:,1024]) x cos_0^2 repairs the difference — no extra
    DFT matmuls for the Nyquist bin at all.
  - top_db clamp applied in linear space (log is monotone): pass 1 writes
    out_pre = (10*log10(max(mel,1e-10)) + 25)/80 and the per-core max of mel;
    after a gpsimd partition_all_reduce + AllReduce(max) across the 8 cores, a
    small fixup pass applies out = max(out_pre, o_thr) with
    o_thr = (10*log10(gmax*1e-8) + 25)/80 — so the collective latency overlaps
    pass-1 output work instead of serializing the whole epilogue.
"""
import sys

sys.path.insert(0, "/opt/trn_rl_repo")

import numpy as np

from concourse import bacc, bass_isa, mybir, tile
from concourse.bass_utils import run_bass_kernel_spmd

dt = mybir.dt
AF = mybir.ActivationFunctionType
ALU = mybir.AluOpType

NCORES = 8
B, T = 32, 221184
WIN, HOP = 2048, 256
FRAMES = (T - WIN) // HOP + 1  # 857
NMEL = 128
BPC = B // NCORES  # 4
UCOLS = T // 256  # 864 columns of 128 per parity
NFC = 8  # f-chunks of 128 (f = 0..1023); f=1024 (Nyquist) handled separately
NH = 16  # k-chunks of 128 (window 2048)
# Second tile overlaps the first by 7 frames so its width is a multiple of 4
# (f32r matmuls reject odd moving-operand widths); overlapped frames are
# recomputed with identical values, so output writes and the max are unaffected.
T_TILES = [(0, 512), (FRAMES - 348, 348)]
C_LOG = 10.0 / float(np.log(10.0))  # 10*log10(x) = C_LOG * ln(x)
AMIN = 1e-10
TOPDB_LIN = 1e-8  # 10**(-80/10)

_compiled = {}


def _build_nc():
    nc = bacc.Bacc(
        "TRN2", target_bir_lowering=False, debug=False, num_devices=NCORES
    )

    xT_d = nc.dram_tensor("xT", [BPC, 128, 2, UCOLS], dt.float32r, kind="ExternalInput")
    wcos_d = nc.dram_tensor(
        "wcos", [128, NFC, NH, 128], dt.float32r, kind="ExternalInput"
    )
    wsin_d = nc.dram_tensor(
        "wsin", [128, NFC, NH, 128], dt.float32r, kind="ExternalInput"
    )
    melT_d = nc.dram_tensor(
        "melT", [128, NFC, NMEL], dt.float32r, kind="ExternalInput"
    )
    melnyq_d = nc.dram_tensor("melnyq", [1, NMEL], dt.float32r, kind="ExternalInput")
    out_d = nc.dram_tensor("out", [BPC, NMEL, FRAMES], dt.float32, kind="ExternalOutput")

    with tile.TileContext(nc) as tc:
        with (
            tc.tile_pool(name="sbw", bufs=1) as sbw,
            tc.tile_pool(name="sbx", bufs=2) as sbx,
            tc.tile_pool(name="sbe", bufs=2) as sbe,
            tc.tile_pool(name="sbf", bufs=4) as sbf,
            tc.tile_pool(name="sbm", bufs=8) as sbm,
            tc.tile_pool(name="ps2", bufs=2, space="PSUM") as ps2,
            tc.tile_pool(name="ps3", bufs=3, space="PSUM") as ps3,
            tc.tile_pool(name="dram", bufs=1, space="DRAM") as dram,
        ):
            # x for batch 0 first (gpsimd DMA queue), then fc=0 weights, then
            # the mel weights, then the rest — so the first DFT matmuls start
            # ~6us in and the remaining weight DMAs stream under compute.
            c2s = []
            for b in range(BPC):
                c2b = sbx.tile([128, 2, UCOLS], dt.float32r, tag="c2", name=f"c2_{b}")
                c2s.append(c2b)
            # parity 0 first: it is all the first (h=0) matmul needs
            nc.gpsimd.dma_start(c2s[0][:, 0], xT_d.ap()[0][:, 0])
            nc.gpsimd.dma_start(c2s[0][:, 1], xT_d.ap()[0][:, 1])
            if BPC > 1:
                nc.gpsimd.dma_start(c2s[1][:], xT_d.ap()[1])

            # Warm up the collective engine while the DFT runs so the real
            # AllReduce at the end starts with rings already configured.
            # Contents are irrelevant (result unused).
            ccw_in = dram.tile([1, 128], dt.float32, name="ccw_in")
            ccw_out = dram.tile([1, 128], dt.float32, name="ccw_out")
            nc.gpsimd.collective_compute(
                "AllReduce",
                ALU.max,
                replica_groups=[list(range(NCORES))],
                ins=[ccw_in[:].opt()],
                outs=[ccw_out[:].opt()],
            )

            wcos_t = []
            wsin_t = []
            for fc in range(NFC):
                ct = sbw.tile([128, NH, 128], dt.float32r, name=f"wcos{fc}")
                st = sbw.tile([128, NH, 128], dt.float32r, name=f"wsin{fc}")
                wcos_t.append(ct)
                wsin_t.append(st)
            # fc=0 weights split across the sync and scalar queues in halves
            # so the first 32 matmuls are never DMA-starved
            nc.sync.dma_start(wcos_t[0][:, 0:8], wcos_d.ap()[:, 0, 0:8])
            nc.scalar.dma_start(wcos_t[0][:, 8:], wcos_d.ap()[:, 0, 8:])
            nc.sync.dma_start(wsin_t[0][:, 0:8], wsin_d.ap()[:, 0, 0:8])
            nc.scalar.dma_start(wsin_t[0][:, 8:], wsin_d.ap()[:, 0, 8:])
            melT_t = sbw.tile([128, NFC, NMEL], dt.float32r, name="melT_t")
            nc.sync.dma_start(melT_t[:], melT_d.ap())
            melnyq_t = sbw.tile([1, NMEL], dt.float32r, name="melnyq_t")
            nc.sync.dma_start(melnyq_t[:], melnyq_d.ap())
            for fc in range(1, NFC):
                nc.sync.dma_start(wcos_t[fc][:], wcos_d.ap()[:, fc])
                nc.sync.dma_start(wsin_t[fc][:], wsin_d.ap()[:, fc])
            for b in range(2, BPC):
                nc.gpsimd.dma_start(c2s[b][:], xT_d.ap()[b])

            prebuf = dram.tile([BPC, NMEL, FRAMES], dt.float32, name="prebuf")
            nslots = BPC * len(T_TILES)
            maxslots = sbw.tile([128, nslots], dt.float32, name="maxslots")

            # ---- pass 1: DFT power + mel + log/affine (no global clamp) ----
            slot = 0
            for b in range(BPC):
                c2 = c2s[b]
                for t0, tt in T_TILES:
                    mel_ps = ps2.tile([128, tt], dt.float32, tag="mel")
                    for fc in range(NFC):
                        cos_ps = ps3.tile([128, tt], dt.float32, tag="cos")
                        sin_ps = ps3.tile([128, tt], dt.float32, tag="sin")
                        for h in range(NH):
                            u0 = t0 + h // 2
                            rhs = c2[:, h % 2, u0 : u0 + tt]
                            nc.tensor.matmul(
                                cos_ps[:], wcos_t[fc][:, h, :], rhs,
                                start=(h == 0), stop=(h == NH - 1),
                                skip_group_check=True,
                            )
                        for h in range(NH):
                            u0 = t0 + h // 2
                            rhs = c2[:, h % 2, u0 : u0 + tt]
                            nc.tensor.matmul(
                                sin_ps[:], wsin_t[fc][:, h, :], rhs,
                                start=(h == 0), stop=(h == NH - 1),
                                skip_group_check=True,
                            )
                        csq = sbe.tile([128, tt], dt.float32r, tag="csq")
                        ssq = sbe.tile([128, tt], dt.float32, tag="ssq")
                        nc.scalar.activation(csq[:], cos_ps[:], AF.Square)
                        nc.scalar.activation(ssq[:], sin_ps[:], AF.Square)
                        if fc == 0:
                            # rank-1 repair of the Nyquist fold (see header)
                            nc.tensor.matmul(
                                mel_ps[:], melnyq_t[:], csq[0:1, :],
                                start=True, stop=False, skip_group_check=True,
                            )
                        stft = sbe.tile([128, tt], dt.float32r, tag="stft")
                        nc.vector.tensor_tensor(stft[:], csq[:], ssq[:], ALU.add)
                        nc.tensor.matmul(
                            mel_ps[:], melT_t[:, fc, :], stft[:],
                            start=False, stop=(fc == NFC - 1),
                            skip_group_check=True,
                        )
                    mel_sb = sbe.tile([128, tt], dt.float32, tag="melsb")
                    nc.vector.tensor_scalar(mel_sb[:], mel_ps[:], AMIN, None, ALU.max)
                    nc.vector.tensor_reduce(
                        maxslots[:, slot : slot + 1], mel_sb[:],
                        mybir.AxisListType.X, ALU.max,
                    )
                    slot += 1
                    nc.scalar.activation(mel_sb[:], mel_sb[:], AF.Ln)
                    nc.vector.tensor_scalar(
                        mel_sb[:], mel_sb[:], C_LOG / 80.0, 25.0 / 80.0,
                        ALU.mult, ALU.add,
                    )
                    nc.scalar.dma_start(prebuf[b, :, t0 : t0 + tt], mel_sb[:])

            # ---- global max across partitions, then across cores ----
            lmax = sbw.tile([128, 1], dt.float32, name="lmax")
            nc.vector.tensor_reduce(
                lmax[:], maxslots[:], mybir.AxisListType.X, ALU.max
            )
            gmax = sbw.tile([128, 1], dt.float32, name="gmax")
            nc.gpsimd.partition_all_reduce(
                gmax[:], lmax[:], channels=128, reduce_op=bass_isa.ReduceOp.max
            )
            cc_in = dram.tile([1, 128], dt.float32, name="cc_in")
            cc_out = dram.tile([1, 128], dt.float32, name="cc_out")
            nc.gpsimd.dma_start(cc_in[:], gmax[:])

            # prefetch fixup inputs on the sync queue while the collective runs
            mts = []
            for b in range(BPC):
                for t0, tt in T_TILES:
                    mt = sbm.tile([128, tt], dt.float32, tag="mt", name=f"mt_{b}_{t0}")
                    nc.sync.dma_start(mt[:], prebuf[b, :, t0 : t0 + tt])
                    mts.append(mt)
            nc.gpsimd.collective_compute(
                "AllReduce",
                ALU.max,
                replica_groups=[list(range(NCORES))],
                ins=[cc_in[:].opt()],
                outs=[cc_out[:].opt()],
            )
            thrsrc = sbw.tile([128, 1], dt.float32, name="thrsrc")
            nc.gpsimd.dma_start(thrsrc[:], cc_out[:])
            # o_thr = (C_LOG*ln(gmax*1e-8) + 25)/80, as a per-partition scalar
            thrlin = sbw.tile([128, 1], dt.float32, name="thrlin")
            nc.vector.tensor_scalar(thrlin[:], thrsrc[:], TOPDB_LIN, None, ALU.mult)
            thrln = sbw.tile([128, 1], dt.float32, name="thrln")
            nc.scalar.activation(thrln[:], thrlin[:], AF.Ln)
            o_thr = sbw.tile([128, 1], dt.float32, name="o_thr")
            nc.vector.tensor_scalar(
                o_thr[:], thrln[:], C_LOG / 80.0, 25.0 / 80.0, ALU.mult, ALU.add
            )

            # ---- pass 2 (fixup): out = max(out_pre, o_thr) ----
            i = 0
            for b in range(BPC):
                for t0, tt in T_TILES:
                    oc = sbf.tile([128, tt], dt.float32, tag="oc")
                    nc.vector.tensor_scalar(oc[:], mts[i][:], o_thr[:], None, ALU.max)
                    eng = (nc.sync, nc.scalar, nc.gpsimd)[i % 3]
                    eng.dma_start(out_d.ap()[b, :, t0 : t0 + tt], oc[:])
                    i += 1

    nc.compile()
    return nc


def _get_nc():
    if "nc" not in _compiled:
        _compiled["nc"] = _build_nc()
    return _compiled["nc"]


def _prep_inputs(x, cos_w, sin_w, mel_w):
    x = np.asarray(x, dtype=np.float32).reshape(B, T)
    wc = np.asarray(cos_w, dtype=np.float32).reshape(WIN // 2 + 1, WIN)  # [1025,2048]
    ws = np.asarray(sin_w, dtype=np.float32).reshape(WIN // 2 + 1, WIN)
    mel = np.asarray(mel_w, dtype=np.float32)  # [128, 1025]

    # x -> [B, 128, 2, 864]: C2[r, par, u] = x[256u + 128par + r]
    xT = np.ascontiguousarray(x.reshape(B, UCOLS, 2, 128).transpose(0, 3, 2, 1))

    def conv_w(w):  # [1024, 2048] -> [128(r), NFC, NH, 128(f within chunk)]
        a = w.reshape(NFC, 128, NH, 128)  # [fc, fi, h, r]
        return np.ascontiguousarray(a.transpose(3, 0, 2, 1))

    wcos = conv_w(wc[:1024])
    # the f=0 sin row is exactly zero; carry the Nyquist cos row there
    ws_mod = ws[:1024].copy()
    ws_mod[0] = wc[1024]
    wsin = conv_w(ws_mod)
    # mel column for f=0 becomes mel_w[:,1024] (applied to cos_0^2 + nyq^2);
    # the rank-1 (mel_w[:,0]-mel_w[:,1024]) x cos_0^2 term repairs it
    mel_mod = mel[:, :1024].copy()
    mel_mod[:, 0] = mel[:, 1024]
    melT = np.ascontiguousarray(
        mel_mod.T.reshape(NFC, 128, NMEL).transpose(1, 0, 2)
    )  # [128, NFC, NMEL]
    melnyq = np.ascontiguousarray((mel[:, 0] - mel[:, 1024])[None, :])  # [1, NMEL]
    return xT, wcos, wsin, melT, melnyq


def kernel(x, cos_w, sin_w, mel_w):
    nc = _get_nc()
    xT, wcos, wsin, melT, melnyq = _prep_inputs(x, cos_w, sin_w, mel_w)
    in_maps = []
    for c in range(NCORES):
        in_maps.append(
            {
                "xT": xT[c * BPC : (c + 1) * BPC],
                "wcos": wcos,
                "wsin": wsin,
                "melT": melT,
                "melnyq": melnyq,
            }
        )
    res = run_bass_kernel_spmd(nc, in_maps, list(range(NCORES)))
    out = np.concatenate([r["out"] for r in res.results], axis=0)  # [32,128,857]
    return out.astype(np.float32)


if __name__ == "__main__":
    rng = np.random.default_rng(0)
    x = rng.standard_normal((B, 1, T), dtype=np.float32)
    wc = rng.standard_normal((1025, 1, WIN), dtype=np.float32)
    wsn = rng.standard_normal((1025, 1, WIN), dtype=np.float32)
    mw = np.abs(rng.standard_normal((NMEL, 1025), dtype=np.float32)).astype(np.float32)
    o = kernel(x, wc, wsn, mw)
    print(o.shape, o.dtype)


# revision 31
# speedup vs baseline: 1.0226x; 1.0226x over previous
"""MelSpectrogramNet on 8 TRN2 NeuronCores (Bass/Tile), data-parallel over batch.

Math (per batch item):
  stft[f,t]  = (sum_k x[256t+k]*wc[f,k])^2 + (sum_k x[256t+k]*ws[f,k])^2
  mel        = mel_w @ stft
  x_db       = 10*log10(max(mel, 1e-10));  x_db = max(x_db, max_all(x_db)-80)
  out        = (x_db + 25) / 80

Device mapping:
  - hop 256 = 2*128, window 2048 = 16*128: x is de-interleaved on the host by
    128-column parity into C2[r, par, u] = x[256u + 128par + r] so the moving
    operand of the DFT matmul for k-chunk h and frame tile [t0, t0+Tt) is the
    CONTIGUOUS slice C2[:, h%2, t0+h//2 : t0+h//2+Tt] (full-rate PE streaming).
  - f32r matmuls (full PE rate, ~1e-4 rel err) for the DFT (16 accumulating
    K=128 matmuls per f-chunk of 128 freqs) and the mel projection.
  - Nyquist bin (f=1024): the f=0 SIN row is exactly zero, so the sin weight
    matrix carries the Nyquist cos row there instead (sin row at Nyquist is
    sin(pi*t) = 0, dropped). Then stft[0] = cos_0^2 + nyq^2; the mel weight
    column for f=0 is swapped to mel_w[:,1024] and a K=1 rank-1 matmul with
    (mel_w[:,0]-mel_w[:,1024]) x cos_0^2 repairs the difference — no extra
    DFT matmuls for the Nyquist bin at all.
  - top_db clamp applied in linear space (log is monotone): pass 1 writes
    out_pre = (10*log10(max(mel,1e-10)) + 25)/80 and the per-core max of mel;
    after a gpsimd partition_all_reduce + AllReduce(max) across the 8 cores, a
    small fixup pass applies out = max(out_pre, o_thr) with
    o_thr = (10*log10(gmax*1e-8) + 25)/80 — so the collective latency overlaps
    pass-1 output work instead of serializing the whole epilogue.
"""
import sys

sys.path.insert(0, "/opt/trn_rl_repo")

import numpy as np

from concourse import bacc, bass_isa, mybir, tile
from concourse.bass_utils import run_bass_kernel_spmd

dt = mybir.dt
AF = mybir.ActivationFunctionType
ALU = mybir.AluOpType

NCORES = 8
B, T = 32, 221184
WIN, HOP = 2048, 256
FRAMES = (T - WIN) // HOP + 1  # 857
NMEL = 128
BPC = B // NCORES  # 4
UCOLS = T // 256  # 864 columns of 128 per parity
NFC = 8  # f-chunks of 128 (f = 0..1023); f=1024 (Nyquist) handled separately
NH = 16  # k-chunks of 128 (window 2048)
# Second tile overlaps the first by 7 frames so its width is a multiple of 4
# (f32r matmuls reject odd moving-operand widths); overlapped frames are
# recomputed with identical values, so output writes and the max are unaffected.
T_TILES = [(0, 512), (FRAMES - 348, 348)]
C_LOG = 10.0 / float(np.log(10.0))  # 10*log10(x) = C_LOG * ln(x)
AMIN = 1e-10
TOPDB_LIN = 1e-8  # 10**(-80/10)

_compiled = {}


def _build_nc():
    nc = bacc.Bacc(
        "TRN2", target_bir_lowering=False, debug=False, num_devices=NCORES
    )

    xT_d = nc.dram_tensor("xT", [BPC, 128, 2, UCOLS], dt.float32r, kind="ExternalInput")
    wcos_d = nc.dram_tensor(
        "wcos", [128, NFC, NH, 128], dt.float32r, kind="ExternalInput"
    )
    wsin_d = nc.dram_tensor(
        "wsin", [128, NFC, NH, 128], dt.float32r, kind="ExternalInput"
    )
    melT_d = nc.dram_tensor(
        "melT", [128, NFC, NMEL], dt.float32r, kind="ExternalInput"
    )
    melnyq_d = nc.dram_tensor("melnyq", [1, NMEL], dt.float32r, kind="ExternalInput")
    out_d = nc.dram_tensor("out", [BPC, NMEL, FRAMES], dt.float32, kind="ExternalOutput")

    with tile.TileContext(nc) as tc:
        with (
            tc.tile_pool(name="sbw", bufs=1) as sbw,
            tc.tile_pool(name="sbx", bufs=2) as sbx,
            tc.tile_pool(name="sbe", bufs=2) as sbe,
            tc.tile_pool(name="sbf", bufs=4) as sbf,
            tc.tile_pool(name="sbm", bufs=8) as sbm,
            tc.tile_pool(name="ps2", bufs=2, space="PSUM") as ps2,
            tc.tile_pool(name="ps3", bufs=3, space="PSUM") as ps3,
            tc.tile_pool(name="dram", bufs=1, space="DRAM") as dram,
        ):
            # x for batch 0 first (gpsimd DMA queue), then fc=0 weights, then
            # the mel weights, then the rest — so the first DFT matmuls start
            # ~6us in and the remaining weight DMAs stream under compute.
            c2s = []
            for b in range(BPC):
                c2b = sbx.tile([128, 2, UCOLS], dt.float32r, tag="c2", name=f"c2_{b}")
                c2s.append(c2b)
            # parity 0 first: it is all the first (h=0) matmul needs
            nc.gpsimd.dma_start(c2s[0][:, 0], xT_d.ap()[0][:, 0])
            nc.gpsimd.dma_start(c2s[0][:, 1], xT_d.ap()[0][:, 1])
            if BPC > 1:
                nc.gpsimd.dma_start(c2s[1][:], xT_d.ap()[1])

            # Warm up the collective engine while the DFT runs so the real
            # AllReduce at the end starts with rings already configured.
            # Contents are irrelevant (result unused).
            ccw_in = dram.tile([1, 128], dt.float32, name="ccw_in")
            ccw_out = dram.tile([1, 128], dt.float32, name="ccw_out")
            nc.gpsimd.collective_compute(
                "AllReduce",
                ALU.max,
                replica_groups=[list(range(NCORES))],
                ins=[ccw_in[:].opt()],
                outs=[ccw_out[:].opt()],
            )

            wcos_t = []
            wsin_t = []
            for fc in range(NFC):
                ct = sbw.tile([128, NH, 128], dt.float32r, name=f"wcos{fc}")
                st = sbw.tile([128, NH, 128], dt.float32r, name=f"wsin{fc}")
                wcos_t.append(ct)
                wsin_t.append(st)
            # fc=0 weights split across the sync and scalar queues in halves
            # so the first 32 matmuls are never DMA-starved
            nc.sync.dma_start(wcos_t[0][:, 0:8], wcos_d.ap()[:, 0, 0:8])
            nc.scalar.dma_start(wcos_t[0][:, 8:], wcos_d.ap()[:, 0, 8:])
            nc.sync.dma_start(wsin_t[0][:, 0:8], wsin_d.ap()[:, 0, 0:8])
            nc.scalar.dma_start(wsin_t[0][:, 8:], wsin_d.ap()[:, 0, 8:])
            melT_t = sbw.tile([128, NFC, NMEL], dt.float32r, name="melT_t")
            nc.sync.dma_start(melT_t[:], melT_d.ap())
            melnyq_t = sbw.tile([1, NMEL], dt.float32r, name="melnyq_t")
            nc.sync.dma_start(melnyq_t[:], melnyq_d.ap())
            for fc in range(1, NFC):
                nc.sync.dma_start(wcos_t[fc][:], wcos_d.ap()[:, fc])
                nc.sync.dma_start(wsin_t[fc][:], wsin_d.ap()[:, fc])
            for b in range(2, BPC):
                nc.gpsimd.dma_start(c2s[b][:], xT_d.ap()[b])

            prebuf = dram.tile([BPC, NMEL, FRAMES], dt.float32, name="prebuf")
            nslots = BPC * len(T_TILES)
            maxslots = sbw.tile([128, nslots], dt.float32, name="maxslots")

            # ---- pass 1: DFT power + mel + log/affine (no global clamp) ----
            slot = 0
            for b in range(BPC):
                c2 = c2s[b]
                for t0, tt in T_TILES:
                    mel_ps = ps2.tile([128, tt], dt.float32, tag="mel")
                    for fc in range(NFC):
                        cos_ps = ps3.tile([128, tt], dt.float32, tag="cos")
                        sin_ps = ps3.tile([128, tt], dt.float32, tag="sin")
                        for h in range(NH):
                            u0 = t0 + h // 2
                            rhs = c2[:, h % 2, u0 : u0 + tt]
                            nc.tensor.matmul(
                                cos_ps[:], wcos_t[fc][:, h, :], rhs,
                                start=(h == 0), stop=(h == NH - 1),
                                skip_group_check=True,
                            )
                        for h in range(NH):
                            u0 = t0 + h // 2
                            rhs = c2[:, h % 2, u0 : u0 + tt]
                            nc.tensor.matmul(
                                sin_ps[:], wsin_t[fc][:, h, :], rhs,
                                start=(h == 0), stop=(h == NH - 1),
                                skip_group_check=True,
                            )
                        csq = sbe.tile([128, tt], dt.float32r, tag="csq")
                        ssq = sbe.tile([128, tt], dt.float32, tag="ssq")
                        nc.scalar.activation(csq[:], cos_ps[:], AF.Square)
                        nc.scalar.activation(ssq[:], sin_ps[:], AF.Square)
                        if fc == 0:
                            # rank-1 repair of the Nyquist fold (see header)
                            nc.tensor.matmul(
                                mel_ps[:], melnyq_t[:], csq[0:1, :],
                                start=True, stop=False, skip_group_check=True,
                            )
                        stft = sbe.tile([128, tt], dt.float32r, tag="stft")
                        nc.vector.tensor_tensor(stft[:], csq[:], ssq[:], ALU.add)
                        nc.tensor.matmul(
                            mel_ps[:], melT_t[:, fc, :], stft[:],
                            start=False, stop=(fc == NFC - 1),
                            skip_group_check=True,
                        )
                    mel_sb = sbe.tile([128, tt], dt.float32, tag="melsb")
                    nc.vector.tensor_scalar(mel_sb[:], mel_ps[:], AMIN, None, ALU.max)
                    nc.vector.tensor_reduce(
                        maxslots[:, slot : slot + 1], mel_sb[:],
                        mybir.AxisListType.X, ALU.max,
                    )
                    slot += 1
                    nc.scalar.activation(mel_sb[:], mel_sb[:], AF.Ln)
                    nc.vector.tensor_scalar(
                        mel_sb[:], mel_sb[:], C_LOG / 80.0, 25.0 / 80.0,
                        ALU.mult, ALU.add,
                    )
                    nc.scalar.dma_start(prebuf[b, :, t0 : t0 + tt], mel_sb[:])

            # ---- global max across partitions, then across cores ----
            lmax = sbw.tile([128, 1], dt.float32, name="lmax")
            nc.vector.tensor_reduce(
                lmax[:], maxslots[:], mybir.AxisListType.X, ALU.max
            )
            gmax = sbw.tile([128, 1], dt.float32, name="gmax")
            nc.gpsimd.partition_all_reduce(
                gmax[:], lmax[:], channels=128, reduce_op=bass_isa.ReduceOp.max
            )
            cc_in = dram.tile([1, 128], dt.float32, name="cc_in")
            cc_out = dram.tile([1, 128], dt.float32, name="cc_out")
            nc.gpsimd.dma_start(cc_in[:], gmax[:])

            # prefetch fixup inputs on the sync queue while the collective runs
            mts = []
            for b in range(BPC):
                for t0, tt in T_TILES:
                    mt = sbm.tile([128, tt], dt.float32, tag="mt", name=f"mt_{b}_{t0}")
                    nc.sync.dma_start(mt[:], prebuf[b, :, t0 : t0 + tt])
                    mts.append(mt)
            nc.gpsimd.collective_compute(
                "AllReduce",
                ALU.max,
                replica_groups=[list(range(NCORES))],
                ins=[cc_in[:].opt()],
                outs=[cc_out[:].opt()],
            )
            thrsrc = sbw.tile([128, 1], dt.float32, name="thrsrc")
            nc.gpsimd.dma_start(thrsrc[:], cc_out[:])
            # o_thr = (C_LOG*ln(gmax*1e-8) + 25)/80, as a per-partition scalar
            thrlin = sbw.tile([128, 1], dt.float32, name="thrlin")
            nc.vector.tensor_scalar(thrlin[:], thrsrc[:], TOPDB_LIN, None, ALU.mult)
            thrln = sbw.tile([128, 1], dt.float32, name="thrln")
            nc.scalar.activation(thrln[:], thrlin[:], AF.Ln)
            o_thr = sbw.tile([128, 1], dt.float32, name="o_thr")
            nc.vector.tensor_scalar(
                o_thr[:], thrln[:], C_LOG / 80.0, 25.0 / 80.0, ALU.mult, ALU.add
            )

            # ---- pass 2 (fixup): out = max(out_pre, o_thr) ----
            i = 0
            for b in range(BPC):
                for t0, tt in T_TILES:
                    oc = sbf.tile([128, tt], dt.float32, tag="oc")
                    nc.vector.tensor_scalar(oc[:], mts[i][:], o_thr[:], None, ALU.max)
                    eng = nc.scalar if i % 2 else nc.sync
                    eng.dma_start(out_d.ap()[b, :, t0 : t0 + tt], oc[:])
                    i += 1

    nc.compile()
    return nc


def _get_nc():
    if "nc" not in _compiled:
        _compiled["nc"] = _build_nc()
    return _compiled["nc"]


def _prep_inputs(x, cos_w, sin_w, mel_w):
    x = np.asarray(x, dtype=np.float32).reshape(B, T)
    wc = np.asarray(cos_w, dtype=np.float32).reshape(WIN // 2 + 1, WIN)  # [1025,2048]
    ws = np.asarray(sin_w, dtype=np.float32).reshape(WIN // 2 + 1, WIN)
    mel = np.asarray(mel_w, dtype=np.float32)  # [128, 1025]

    # x -> [B, 128, 2, 864]: C2[r, par, u] = x[256u + 128par + r]
    xT = np.ascontiguousarray(x.reshape(B, UCOLS, 2, 128).transpose(0, 3, 2, 1))

    def conv_w(w):  # [1024, 2048] -> [128(r), NFC, NH, 128(f within chunk)]
        a = w.reshape(NFC, 128, NH, 128)  # [fc, fi, h, r]
        return np.ascontiguousarray(a.transpose(3, 0, 2, 1))

    wcos = conv_w(wc[:1024])
    # the f=0 sin row is exactly zero; carry the Nyquist cos row there
    ws_mod = ws[:1024].copy()
    ws_mod[0] = wc[1024]
    wsin = conv_w(ws_mod)
    # mel column for f=0 becomes mel_w[:,1024] (applied to cos_0^2 + nyq^2);
    # the rank-1 (mel_w[:,0]-mel_w[:,1024]) x cos_0^2 term repairs it
    mel_mod = mel[:, :1024].copy()
    mel_mod[:, 0] = mel[:, 1024]
    melT = np.ascontiguousarray(
        mel_mod.T.reshape(NFC, 128, NMEL).transpose(1, 0, 2)
    )  # [128, NFC, NMEL]
    melnyq = np.ascontiguousarray((mel[:, 0] - mel[:, 1024])[None, :])  # [1, NMEL]
    return xT, wcos, wsin, melT, melnyq


def kernel(x, cos_w, sin_w, mel_w):
    nc = _get_nc()
    xT, wcos, wsin, melT, melnyq = _prep_inputs(x, cos_w, sin_w, mel_w)
    in_maps = []
    for c in range(NCORES):
        in_maps.append(
            {
                "xT": xT[c * BPC : (c + 1) * BPC],
                "wcos": wcos,
                "wsin": wsin,
                "melT": melT,
                "melnyq": melnyq,
            }
        )
    res = run_bass_kernel_spmd(nc, in_maps, list(range(NCORES)))
    out = np.concatenate([r["out"] for r in res.results], axis=0)  # [32,128,857]
    return out.astype(np.float32)


if __name__ == "__main__":
    rng = np.random.default_rng(0)
    x = rng.standard_normal((B, 1, T), dtype=np.float32)
    wc = rng.standard_normal((1025, 1, WIN), dtype=np.float32)
    wsn = rng.standard_normal((1025, 1, WIN), dtype=np.float32)
    mw = np.abs(rng.standard_normal((NMEL, 1025), dtype=np.float32)).astype(np.float32)
    o = kernel(x, wc, wsn, mw)
    print(o.shape, o.dtype)
